# revision 6
# baseline (speedup 1.0000x reference)
"""SAN aggregation kernel for Trainium2 (Bass/Tile), 8-core data-parallel.

Problem: out[n,c,h,w] = sum_k w[n, c//8, k, h*W+w] * xpad[n, c, h+dh(k), w+dw(k)]
  x: [8, 64, 128, 128] f32, w: [8, 8, 9, 16384] f32, 3x3 window, pad 1.

Sharding: batch dim N=8 across 8 NeuronCores (1 image per core).

v3 design:
  - The host pre-packs both inputs into the exact fp16 SBUF layout
    (incl. zero halo rows/cols), so every DMA is a plain contiguous
    partition-strided copy and DRAM traffic is halved vs f32.
  - DVE computes ONLY the 9 per-tap products (tensor_mul in the fp16
    2x perf mode); tap SUMMING runs on the otherwise-idle PE: an
    identity [128,128] stationary matmul accumulates the 9 product
    tensors into PSUM f32 (start=k==0 / stop=k==8 per 512-col chunk).
  - ACT (also idle) evicts PSUM f32 -> SBUF fp16 per half-group; the
    stores ride the gpsimd SWDGE queue after all loads; host unpacks.
  - Ramp: the gpsimd SWDGE queue only starts descriptor generation at
    ~7.8us (framework preamble) and serializes ~0.67us per DMA, so the
    first working set (ident, w tap 0, x gl 0 / gl 1) rides the Sync
    and Scalar engines' hardware-DGE queues instead, which are ready
    right after their (shorter) preambles.
  This cuts DVE busy from ~17 passes (~82us) to ~9 passes (~45us),
  with PE/ACT/DMA all hidden behind it.
"""

import sys
import os

for _p in ("/opt/trn_rl_repo", "/root/.axon_site/_ro/trn_rl_repo"):
    if _p not in sys.path and os.path.isdir(_p):
        sys.path.append(_p)

import numpy as np

import concourse.bass as bass
import concourse.bacc as bacc
import concourse.mybir as mybir
import bass_rust
from concourse.tile import TileContext
from concourse.tile_rust import add_dep_helper

F32 = mybir.dt.float32
F16 = mybir.dt.float16

C, H, W = 64, 128, 128
S = H * W          # 16384
CW, GL = 8, 8      # weight channels, share planes
HB = 16            # row blocks
RB = H // HB       # rows per block = 8
XR = RB + 2        # 10 rows incl halo
XW = W + 2         # 130 cols incl left/right zero pad
XGL = XR * XW      # 1300 elements per gl block in x16
SB = RB * W        # 1024 output elems per partition per gl
NG = 2             # gls per compute group
NGRP = GL // NG    # 4 groups
CH = 512           # matmul moving-dim chunk (hw max)


def _ap(base, dims, extra_offset=0):
    """Copy AP `base`, replace its [step,count] dims, bump offset.

    dims[0] is the partition dim: step "P" substitutes the base AP's own
    partition stride (flat element space, = free width).
    """
    c = base.copy()
    pstep = base.ap[0][0]
    dims = [[pstep if s == "P" else s, n] for s, n in dims]
    c.ap = bass_rust.VecI64Pair(dims)
    if extra_offset:
        c.offset = c.offset + extra_offset
    return c


def build_program():
    nc = bacc.Bacc("TRN2", target_bir_lowering=False, debug=False)
    x_d = nc.dram_tensor("x", [128, GL * XGL], F16, kind="ExternalInput")
    w_d = nc.dram_tensor("w", [128, 9 * SB], F16, kind="ExternalInput")
    o_d = nc.dram_tensor("out", [128, GL * SB], F16, kind="ExternalOutput")
    id_d = nc.inline_tensor(np.eye(128, dtype=np.float16), name="ident")

    with TileContext(nc) as tc:
        with tc.tile_pool(name="main", bufs=1) as pool, \
             tc.tile_pool(name="tmps", bufs=4) as tpool, \
             tc.tile_pool(name="evs", bufs=4) as epool, \
             tc.tile_pool(name="ps", bufs=2, space="PSUM") as ppool:
            x16 = pool.tile([128, GL * XGL], F16)
            w16 = pool.tile([128, 9 * SB], F16)
            ident = pool.tile([128, 128], F16)

            # Per-engine program-order pins: the static scheduler
            # reorders same-engine instructions by its own cost model;
            # chain them so issue order == consumption order.
            _prev = {}

            def _pin(eng, d):
                if eng in _prev:
                    add_dep_helper(d.ins, _prev[eng].ins, sync=False,
                                   reason="issue order")
                _prev[eng] = d
                return d

            ENG = {"pool": nc.gpsimd, "sync": nc.sync, "act": nc.scalar}

            def load_ident(q):
                _pin(q, ENG[q].dma_start(
                    out=_ap(ident[:], [["P", 128], [1, 128]]),
                    in_=_ap(id_d.ap(), [[128, 128], [1, 128]])))

            def load_w(q, k0, nk):
                _pin(q, ENG[q].dma_start(
                    out=_ap(w16[:], [["P", 128], [1, nk * SB]],
                            extra_offset=k0 * SB),
                    in_=_ap(w_d.ap(), [[9 * SB, 128], [1, nk * SB]],
                            extra_offset=k0 * SB)))

            def load_x(q, g0, n):
                _pin(q, ENG[q].dma_start(
                    out=_ap(x16[:], [["P", 128], [1, n * XGL]],
                            extra_offset=g0 * XGL),
                    in_=_ap(x_d.ap(), [[GL * XGL, 128], [1, n * XGL]],
                            extra_offset=g0 * XGL)))

            # The gpsimd SWDGE queue is the fast one (~300 GB/s once
            # flowing) but starts descgen only at ~7.8us and serializes
            # ~0.65us/DMA; the Sync/Scalar HWDGE queues are slow
            # (~70-160 GB/s) but independent and ready ~6.5us.  So the
            # ramp-critical stream (w tap 0, x gl 0-1, then w taps 1-8
            # just ahead of group 0's consumption) rides SWDGE; the
            # identity and the later x pairs ride the side queues.
            load_ident("sync")
            load_w("pool", 0, 1)
            load_x("pool", 0, 2)
            load_w("pool", 1, 1)
            load_w("pool", 2, 1)
            load_w("pool", 3, 1)
            load_w("pool", 4, 1)
            load_w("pool", 5, 2)
            load_w("pool", 7, 2)
            load_x("act", 2, 2)
            load_x("sync", 4, 2)
            load_x("act", 6, 2)

            def out_dma(gl, src):
                """Store one gl from fp16 SBUF -> fp16 DRAM (SWDGE)."""
                return _pin("pool", nc.gpsimd.dma_start(
                    out=_ap(o_d.ap(), [[GL * SB, 128], [1, SB]],
                            extra_offset=gl * SB),
                    in_=_ap(src[:], [["P", 128], [1, SB]])))

            # tap (dh, dw): prod[h', w] = w_k[h', w] * x[r=h'+dh, c'=w+dw]
            # (the x col pads make the dw=0 / dw=2 borders exact zeros).
            for g in range(NGRP):
                g0 = g * NG
                ps = ppool.tile([128, NG * SB], F32, tag="ps", name="ps")
                for k in range(9):
                    dh, dw = divmod(k, 3)
                    t = tpool.tile([128, NG * SB], F16, tag="t", name="t")
                    xv = _ap(x16[:], [["P", 128], [XGL, NG], [XW, RB],
                                      [1, W]],
                             extra_offset=g0 * XGL + dh * XW + dw)
                    wv = _ap(w16[:], [["P", 128], [0, NG], [W, RB],
                                      [1, W]],
                             extra_offset=k * SB)
                    tv = _ap(t[:], [["P", 128], [SB, NG], [W, RB],
                                    [1, W]])
                    _pin("dve", nc.vector.tensor_mul(out=tv, in0=xv,
                                                     in1=wv))
                    for cc in range(NG * SB // CH):
                        _pin("pe", nc.tensor.matmul(
                            out=_ap(ps[:], [["P", 128], [1, CH]],
                                    extra_offset=cc * CH),
                            lhsT=ident[:],
                            rhs=_ap(t[:], [["P", 128], [1, CH]],
                                    extra_offset=cc * CH),
                            start=(k == 0), stop=(k == 8)))
                # evict PSUM -> fp16 SBUF per gl on ACT (DMA cannot
                # read PSUM).  In the last group, gl 7 evicts on the
                # by-then-idle DVE so the two final evictions overlap.
                for h in range(NG):
                    ev = epool.tile([128, SB], F16, tag="ev", name="ev")
                    pv = _ap(ps[:], [["P", 128], [1, SB]],
                             extra_offset=h * SB)
                    if g == NGRP - 1 and h == NG - 1:
                        _pin("dve", nc.vector.tensor_copy(out=ev[:],
                                                          in_=pv))
                    else:
                        _pin("act", nc.scalar.copy(out=ev[:], in_=pv))
                    out_dma(g0 + h, ev)

    nc.compile()
    return nc


_NC_CACHE = None


def _get_nc():
    global _NC_CACHE
    if _NC_CACHE is None:
        _NC_CACHE = build_program()
    return _NC_CACHE


def pack_inputs(x, w):
    """x: [N,64,128,128] f32, w: [N,8,9,16384] f32 ->
    xp: [N,128,10400] f16, wp: [N,128,9216] f16 (per-core SBUF images)."""
    N = x.shape[0]
    xq = np.zeros((N, C, H + 2, W + 2), np.float16)
    xq[:, :, 1:H + 1, 1:W + 1] = x
    # [N, hb, cw, gl, r, col]
    xp = np.empty((N, HB, CW, GL, XR, XW), np.float16)
    xv = xq.reshape(N, CW, GL, H + 2, XW)
    for hb in range(HB):
        xp[:, hb] = xv[:, :, :, hb * RB:hb * RB + XR, :]
    wp = np.asarray(w, np.float16).reshape(N, CW, 9, HB, SB).transpose(
        0, 3, 1, 2, 4)  # [N, hb, cw, k, sb]
    return (np.ascontiguousarray(xp.reshape(N, 128, GL * XGL)),
            np.ascontiguousarray(wp.reshape(N, 128, 9 * SB)))


def unpack_output(o):
    """o: [N,128,8192] f16 -> [N,64,128,128] f32."""
    N = o.shape[0]
    v = o.reshape(N, HB, CW, GL, RB, W).transpose(0, 2, 3, 1, 4, 5)
    return np.ascontiguousarray(v.reshape(N, C, H, W)).astype(np.float32)


def kernel(input, weight):
    """input: [8,64,128,128] f32, weight: [8,8,9,16384] f32 ->
    [8,64,128,128] f32."""
    from concourse.bass_utils import run_bass_kernel_spmd

    x = np.asarray(input, dtype=np.float32)
    w = np.asarray(weight, dtype=np.float32)
    N = x.shape[0]
    xp, wp = pack_inputs(x, w)
    nc = _get_nc()
    in_maps = [{"x": xp[i], "w": wp[i]} for i in range(N)]
    res = run_bass_kernel_spmd(nc, in_maps, core_ids=list(range(N)))
    o = np.stack([res.results[i]["out"] for i in range(N)])
    return unpack_output(o)


# revision 7
# speedup vs baseline: 1.0911x; 1.0911x over previous
"""SAN aggregation kernel for Trainium2 (Bass/Tile), 8-core data-parallel.

Problem: out[n,c,h,w] = sum_k w[n, c//8, k, h*W+w] * xpad[n, c, h+dh(k), w+dw(k)]
  x: [8, 64, 128, 128] f32, w: [8, 8, 9, 16384] f32, 3x3 window, pad 1.

Sharding: batch dim N=8 across 8 NeuronCores (1 image per core).

v3 design:
  - The host pre-packs both inputs into the exact fp16 SBUF layout
    (incl. zero halo rows/cols), so every DMA is a plain contiguous
    partition-strided copy and DRAM traffic is halved vs f32.
  - DVE computes ONLY the 9 per-tap products (tensor_mul in the fp16
    2x perf mode); tap SUMMING runs on the otherwise-idle PE: an
    identity [128,128] stationary matmul accumulates the 9 product
    tensors into PSUM f32 (start=k==0 / stop=k==8 per 512-col chunk).
  - ACT (also idle) evicts PSUM f32 -> SBUF fp16 per half-group; the
    stores ride the gpsimd SWDGE queue after all loads; host unpacks.
  - Ramp: the gpsimd SWDGE queue only starts descriptor generation at
    ~7.8us (framework preamble) and serializes ~0.67us per DMA, so the
    first working set (ident, w tap 0, x gl 0 / gl 1) rides the Sync
    and Scalar engines' hardware-DGE queues instead, which are ready
    right after their (shorter) preambles.
  This cuts DVE busy from ~17 passes (~82us) to ~9 passes (~45us),
  with PE/ACT/DMA all hidden behind it.
"""

import sys
import os

for _p in ("/opt/trn_rl_repo", "/root/.axon_site/_ro/trn_rl_repo"):
    if _p not in sys.path and os.path.isdir(_p):
        sys.path.append(_p)

import numpy as np

import concourse.bass as bass
import concourse.bacc as bacc
import concourse.mybir as mybir
import bass_rust
from concourse.tile import TileContext
from concourse.tile_rust import add_dep_helper

F32 = mybir.dt.float32
F16 = mybir.dt.float16

C, H, W = 64, 128, 128
S = H * W          # 16384
CW, GL = 8, 8      # weight channels, share planes
HB = 16            # row blocks
RB = H // HB       # rows per block = 8
XR = RB + 2        # 10 rows incl halo
XW = W + 2         # 130 cols incl left/right zero pad
XGL = XR * XW      # 1300 elements per gl block in x16
SB = RB * W        # 1024 output elems per partition per gl
NG = 2             # gls per compute group
NGRP = GL // NG    # 4 groups
CH = 512           # matmul moving-dim chunk (hw max)


def _ap(base, dims, extra_offset=0):
    """Copy AP `base`, replace its [step,count] dims, bump offset.

    dims[0] is the partition dim: step "P" substitutes the base AP's own
    partition stride (flat element space, = free width).
    """
    c = base.copy()
    pstep = base.ap[0][0]
    dims = [[pstep if s == "P" else s, n] for s, n in dims]
    c.ap = bass_rust.VecI64Pair(dims)
    if extra_offset:
        c.offset = c.offset + extra_offset
    return c


def build_program():
    nc = bacc.Bacc("TRN2", target_bir_lowering=False, debug=False)
    x_d = nc.dram_tensor("x", [128, GL * XGL], F16, kind="ExternalInput")
    w_d = nc.dram_tensor("w", [128, 9 * SB], F16, kind="ExternalInput")
    o_d = nc.dram_tensor("out", [128, GL * SB], F16, kind="ExternalOutput")
    id_d = nc.inline_tensor(np.eye(128, dtype=np.float16), name="ident")

    with TileContext(nc) as tc:
        with tc.tile_pool(name="main", bufs=1) as pool, \
             tc.tile_pool(name="tmps", bufs=4) as tpool, \
             tc.tile_pool(name="evs", bufs=4) as epool, \
             tc.tile_pool(name="ps", bufs=2, space="PSUM") as ppool:
            x16 = pool.tile([128, GL * XGL], F16)
            w16 = pool.tile([128, 9 * SB], F16)
            ident = pool.tile([128, 128], F16)

            # Per-engine program-order pins: the static scheduler
            # reorders same-engine instructions by its own cost model;
            # chain them so issue order == consumption order.
            _prev = {}

            def _pin(eng, d):
                if eng in _prev:
                    add_dep_helper(d.ins, _prev[eng].ins, sync=False,
                                   reason="issue order")
                _prev[eng] = d
                return d

            ENG = {"pool": nc.gpsimd, "sync": nc.sync, "act": nc.scalar}

            def load_ident(q):
                _pin(q, ENG[q].dma_start(
                    out=_ap(ident[:], [["P", 128], [1, 128]]),
                    in_=_ap(id_d.ap(), [[128, 128], [1, 128]])))

            def load_w(q, k0, nk):
                _pin(q, ENG[q].dma_start(
                    out=_ap(w16[:], [["P", 128], [1, nk * SB]],
                            extra_offset=k0 * SB),
                    in_=_ap(w_d.ap(), [[9 * SB, 128], [1, nk * SB]],
                            extra_offset=k0 * SB)))

            def load_x(q, g0, n):
                _pin(q, ENG[q].dma_start(
                    out=_ap(x16[:], [["P", 128], [1, n * XGL]],
                            extra_offset=g0 * XGL),
                    in_=_ap(x_d.ap(), [[GL * XGL, 128], [1, n * XGL]],
                            extra_offset=g0 * XGL)))

            # ALL bulk loads ride the single gpsimd SWDGE queue (it
            # alone sustains ~300 GB/s; concurrent HWDGE queues were
            # measured to steal DRAM-channel bandwidth from it and made
            # the ramp WORSE).  Order: w tap 0 + x gl 0-1 (first mul),
            # then w taps 1-8 racing just ahead of group 0's ~1.14us/tap
            # consumption, then the x pairs (needed ~10us/group later).
            # Only the tiny identity rides the sync HWDGE queue.
            load_ident("sync")
            load_w("pool", 0, 1)
            load_x("pool", 0, 2)
            load_w("pool", 1, 1)
            load_w("pool", 2, 1)
            load_w("pool", 3, 1)
            load_w("pool", 4, 1)
            load_w("pool", 5, 1)
            load_w("pool", 6, 1)
            load_w("pool", 7, 1)
            load_w("pool", 8, 1)
            load_x("pool", 2, 2)
            load_x("pool", 4, 2)
            load_x("pool", 6, 2)

            def out_dma(gl, src):
                """Store one gl from fp16 SBUF -> fp16 DRAM (SWDGE)."""
                return _pin("pool", nc.gpsimd.dma_start(
                    out=_ap(o_d.ap(), [[GL * SB, 128], [1, SB]],
                            extra_offset=gl * SB),
                    in_=_ap(src[:], [["P", 128], [1, SB]])))

            # tap (dh, dw): prod[h', w] = w_k[h', w] * x[r=h'+dh, c'=w+dw]
            # (the x col pads make the dw=0 / dw=2 borders exact zeros).
            for g in range(NGRP):
                g0 = g * NG
                ps = ppool.tile([128, NG * SB], F32, tag="ps", name="ps")
                for k in range(9):
                    dh, dw = divmod(k, 3)
                    t = tpool.tile([128, NG * SB], F16, tag="t", name="t")
                    xv = _ap(x16[:], [["P", 128], [XGL, NG], [XW, RB],
                                      [1, W]],
                             extra_offset=g0 * XGL + dh * XW + dw)
                    wv = _ap(w16[:], [["P", 128], [0, NG], [W, RB],
                                      [1, W]],
                             extra_offset=k * SB)
                    tv = _ap(t[:], [["P", 128], [SB, NG], [W, RB],
                                    [1, W]])
                    _pin("dve", nc.vector.tensor_mul(out=tv, in0=xv,
                                                     in1=wv))
                    for cc in range(NG * SB // CH):
                        _pin("pe", nc.tensor.matmul(
                            out=_ap(ps[:], [["P", 128], [1, CH]],
                                    extra_offset=cc * CH),
                            lhsT=ident[:],
                            rhs=_ap(t[:], [["P", 128], [1, CH]],
                                    extra_offset=cc * CH),
                            start=(k == 0), stop=(k == 8)))
                # evict PSUM -> fp16 SBUF on ACT (DMA cannot read
                # PSUM).  Last group: gl 6 on ACT and gl 7 on the
                # by-then-idle DVE in parallel, into ONE tile, stored
                # with a single descriptor to shorten the tail.
                if g == NGRP - 1:
                    ev = epool.tile([128, NG * SB], F16, tag="ev2",
                                    name="ev2")
                    _pin("act", nc.scalar.copy(
                        out=_ap(ev[:], [["P", 128], [1, SB]]),
                        in_=_ap(ps[:], [["P", 128], [1, SB]])))
                    _pin("dve", nc.vector.tensor_copy(
                        out=_ap(ev[:], [["P", 128], [1, SB]],
                                extra_offset=SB),
                        in_=_ap(ps[:], [["P", 128], [1, SB]],
                                extra_offset=SB)))
                    _pin("pool", nc.gpsimd.dma_start(
                        out=_ap(o_d.ap(), [[GL * SB, 128], [1, NG * SB]],
                                extra_offset=g0 * SB),
                        in_=_ap(ev[:], [["P", 128], [1, NG * SB]])))
                else:
                    for h in range(NG):
                        ev = epool.tile([128, SB], F16, tag="ev",
                                        name="ev")
                        _pin("act", nc.scalar.copy(
                            out=ev[:],
                            in_=_ap(ps[:], [["P", 128], [1, SB]],
                                    extra_offset=h * SB)))
                        out_dma(g0 + h, ev)

    nc.compile()
    return nc


_NC_CACHE = None


def _get_nc():
    global _NC_CACHE
    if _NC_CACHE is None:
        _NC_CACHE = build_program()
    return _NC_CACHE


def pack_inputs(x, w):
    """x: [N,64,128,128] f32, w: [N,8,9,16384] f32 ->
    xp: [N,128,10400] f16, wp: [N,128,9216] f16 (per-core SBUF images)."""
    N = x.shape[0]
    xq = np.zeros((N, C, H + 2, W + 2), np.float16)
    xq[:, :, 1:H + 1, 1:W + 1] = x
    # [N, hb, cw, gl, r, col]
    xp = np.empty((N, HB, CW, GL, XR, XW), np.float16)
    xv = xq.reshape(N, CW, GL, H + 2, XW)
    for hb in range(HB):
        xp[:, hb] = xv[:, :, :, hb * RB:hb * RB + XR, :]
    wp = np.asarray(w, np.float16).reshape(N, CW, 9, HB, SB).transpose(
        0, 3, 1, 2, 4)  # [N, hb, cw, k, sb]
    return (np.ascontiguousarray(xp.reshape(N, 128, GL * XGL)),
            np.ascontiguousarray(wp.reshape(N, 128, 9 * SB)))


def unpack_output(o):
    """o: [N,128,8192] f16 -> [N,64,128,128] f32."""
    N = o.shape[0]
    v = o.reshape(N, HB, CW, GL, RB, W).transpose(0, 2, 3, 1, 4, 5)
    return np.ascontiguousarray(v.reshape(N, C, H, W)).astype(np.float32)


def kernel(input, weight):
    """input: [8,64,128,128] f32, weight: [8,8,9,16384] f32 ->
    [8,64,128,128] f32."""
    from concourse.bass_utils import run_bass_kernel_spmd

    x = np.asarray(input, dtype=np.float32)
    w = np.asarray(weight, dtype=np.float32)
    N = x.shape[0]
    xp, wp = pack_inputs(x, w)
    nc = _get_nc()
    in_maps = [{"x": xp[i], "w": wp[i]} for i in range(N)]
    res = run_bass_kernel_spmd(nc, in_maps, core_ids=list(range(N)))
    o = np.stack([res.results[i]["out"] for i in range(N)])
    return unpack_output(o)


# revision 8
# speedup vs baseline: 1.1042x; 1.0120x over previous
"""SAN aggregation kernel for Trainium2 (Bass/Tile), 8-core data-parallel.

Problem: out[n,c,h,w] = sum_k w[n, c//8, k, h*W+w] * xpad[n, c, h+dh(k), w+dw(k)]
  x: [8, 64, 128, 128] f32, w: [8, 8, 9, 16384] f32, 3x3 window, pad 1.

Sharding: batch dim N=8 across 8 NeuronCores (1 image per core).

v3 design:
  - The host pre-packs both inputs into the exact fp16 SBUF layout
    (incl. zero halo rows/cols), so every DMA is a plain contiguous
    partition-strided copy and DRAM traffic is halved vs f32.
  - DVE computes ONLY the 9 per-tap products (tensor_mul in the fp16
    2x perf mode); tap SUMMING runs on the otherwise-idle PE: an
    identity [128,128] stationary matmul accumulates the 9 product
    tensors into PSUM f32 (start=k==0 / stop=k==8 per 512-col chunk).
  - ACT (also idle) evicts PSUM f32 -> SBUF fp16 per half-group; the
    stores ride the gpsimd SWDGE queue after all loads; host unpacks.
  - Ramp: the gpsimd SWDGE queue only starts descriptor generation at
    ~7.8us (framework preamble) and serializes ~0.67us per DMA, so the
    first working set (ident, w tap 0, x gl 0 / gl 1) rides the Sync
    and Scalar engines' hardware-DGE queues instead, which are ready
    right after their (shorter) preambles.
  This cuts DVE busy from ~17 passes (~82us) to ~9 passes (~45us),
  with PE/ACT/DMA all hidden behind it.
"""

import sys
import os

for _p in ("/opt/trn_rl_repo", "/root/.axon_site/_ro/trn_rl_repo"):
    if _p not in sys.path and os.path.isdir(_p):
        sys.path.append(_p)

import numpy as np

import concourse.bass as bass
import concourse.bacc as bacc
import concourse.mybir as mybir
import bass_rust
from concourse.tile import TileContext
from concourse.tile_rust import add_dep_helper

F32 = mybir.dt.float32
F16 = mybir.dt.float16

C, H, W = 64, 128, 128
S = H * W          # 16384
CW, GL = 8, 8      # weight channels, share planes
HB = 16            # row blocks
RB = H // HB       # rows per block = 8
XR = RB + 2        # 10 rows incl halo
XW = W + 2         # 130 cols incl left/right zero pad
XGL = XR * XW      # 1300 elements per gl block in x16
SB = RB * W        # 1024 output elems per partition per gl
NG = 2             # gls per compute group
NGRP = GL // NG    # 4 groups
CH = 512           # matmul moving-dim chunk (hw max)


def _ap(base, dims, extra_offset=0):
    """Copy AP `base`, replace its [step,count] dims, bump offset.

    dims[0] is the partition dim: step "P" substitutes the base AP's own
    partition stride (flat element space, = free width).
    """
    c = base.copy()
    pstep = base.ap[0][0]
    dims = [[pstep if s == "P" else s, n] for s, n in dims]
    c.ap = bass_rust.VecI64Pair(dims)
    if extra_offset:
        c.offset = c.offset + extra_offset
    return c


def build_program():
    nc = bacc.Bacc("TRN2", target_bir_lowering=False, debug=False)
    x_d = nc.dram_tensor("x", [128, GL * XGL], F16, kind="ExternalInput")
    w_d = nc.dram_tensor("w", [128, 9 * SB], F16, kind="ExternalInput")
    o_d = nc.dram_tensor("out", [128, GL * SB], F16, kind="ExternalOutput")
    id_d = nc.inline_tensor(np.eye(128, dtype=np.float16), name="ident")

    with TileContext(nc) as tc:
        with tc.tile_pool(name="main", bufs=1) as pool, \
             tc.tile_pool(name="tmps", bufs=4) as tpool, \
             tc.tile_pool(name="evs", bufs=4) as epool, \
             tc.tile_pool(name="ps", bufs=2, space="PSUM") as ppool:
            x16 = pool.tile([128, GL * XGL], F16)
            w16 = pool.tile([128, 9 * SB], F16)
            ident = pool.tile([128, 128], F16)

            # Per-engine program-order pins: the static scheduler
            # reorders same-engine instructions by its own cost model;
            # chain them so issue order == consumption order.
            _prev = {}

            def _pin(eng, d):
                if eng in _prev:
                    add_dep_helper(d.ins, _prev[eng].ins, sync=False,
                                   reason="issue order")
                _prev[eng] = d
                return d

            ENG = {"pool": nc.gpsimd, "sync": nc.sync, "act": nc.scalar}

            def load_ident(q):
                _pin(q, ENG[q].dma_start(
                    out=_ap(ident[:], [["P", 128], [1, 128]]),
                    in_=_ap(id_d.ap(), [[128, 128], [1, 128]])))

            def load_w(q, k0, nk):
                _pin(q, ENG[q].dma_start(
                    out=_ap(w16[:], [["P", 128], [1, nk * SB]],
                            extra_offset=k0 * SB),
                    in_=_ap(w_d.ap(), [[9 * SB, 128], [1, nk * SB]],
                            extra_offset=k0 * SB)))

            def load_x(q, g0, n):
                _pin(q, ENG[q].dma_start(
                    out=_ap(x16[:], [["P", 128], [1, n * XGL]],
                            extra_offset=g0 * XGL),
                    in_=_ap(x_d.ap(), [[GL * XGL, 128], [1, n * XGL]],
                            extra_offset=g0 * XGL)))

            # ALL bulk loads ride the single gpsimd SWDGE queue (it
            # alone sustains ~300 GB/s; concurrent HWDGE queues were
            # measured to steal DRAM-channel bandwidth from it and made
            # the ramp WORSE).  Order: w tap 0 + x gl 0-1 (first mul),
            # then w taps 1-8 racing just ahead of group 0's ~1.14us/tap
            # consumption, then the x pairs (needed ~10us/group later).
            # Only the tiny identity rides the sync HWDGE queue.
            load_ident("sync")
            load_w("pool", 0, 1)
            load_x("pool", 0, 2)
            load_w("pool", 1, 1)
            load_w("pool", 2, 1)
            load_w("pool", 3, 1)
            load_w("pool", 4, 1)
            load_w("pool", 5, 1)
            load_w("pool", 6, 1)
            load_w("pool", 7, 1)
            load_w("pool", 8, 1)
            load_x("pool", 2, 2)
            load_x("pool", 4, 2)
            load_x("pool", 6, 2)

            def out_dma(gl, src):
                """Store one gl from fp16 SBUF -> fp16 DRAM (SWDGE)."""
                return _pin("pool", nc.gpsimd.dma_start(
                    out=_ap(o_d.ap(), [[GL * SB, 128], [1, SB]],
                            extra_offset=gl * SB),
                    in_=_ap(src[:], [["P", 128], [1, SB]])))

            # tap (dh, dw): prod[h', w] = w_k[h', w] * x[r=h'+dh, c'=w+dw]
            # (the x col pads make the dw=0 / dw=2 borders exact zeros).
            for g in range(NGRP):
                g0 = g * NG
                ps = ppool.tile([128, NG * SB], F32, tag="ps", name="ps")
                for k in range(9):
                    dh, dw = divmod(k, 3)
                    t = tpool.tile([128, NG * SB], F16, tag="t", name="t")
                    xv = _ap(x16[:], [["P", 128], [XGL, NG], [XW, RB],
                                      [1, W]],
                             extra_offset=g0 * XGL + dh * XW + dw)
                    wv = _ap(w16[:], [["P", 128], [0, NG], [W, RB],
                                      [1, W]],
                             extra_offset=k * SB)
                    tv = _ap(t[:], [["P", 128], [SB, NG], [W, RB],
                                    [1, W]])
                    _pin("dve", nc.vector.tensor_mul(out=tv, in0=xv,
                                                     in1=wv))
                    for cc in range(NG * SB // CH):
                        _pin("pe", nc.tensor.matmul(
                            out=_ap(ps[:], [["P", 128], [1, CH]],
                                    extra_offset=cc * CH),
                            lhsT=ident[:],
                            rhs=_ap(t[:], [["P", 128], [1, CH]],
                                    extra_offset=cc * CH),
                            start=(k == 0), stop=(k == 8)))
                # evict PSUM -> fp16 SBUF on ACT (DMA cannot read
                # PSUM); separate tiles per gl so deps stay chunk-level.
                # Last group: gl 7 evicts on the by-then-idle DVE in
                # parallel with ACT's gl 6.
                for h in range(NG):
                    ev = epool.tile([128, SB], F16, tag="ev", name="ev")
                    pv = _ap(ps[:], [["P", 128], [1, SB]],
                             extra_offset=h * SB)
                    if g == NGRP - 1 and h == NG - 1:
                        _pin("dve", nc.vector.tensor_copy(out=ev[:],
                                                          in_=pv))
                    else:
                        _pin("act", nc.scalar.copy(out=ev[:], in_=pv))
                    out_dma(g0 + h, ev)

    nc.compile()
    return nc


_NC_CACHE = None


def _get_nc():
    global _NC_CACHE
    if _NC_CACHE is None:
        _NC_CACHE = build_program()
    return _NC_CACHE


def pack_inputs(x, w):
    """x: [N,64,128,128] f32, w: [N,8,9,16384] f32 ->
    xp: [N,128,10400] f16, wp: [N,128,9216] f16 (per-core SBUF images)."""
    N = x.shape[0]
    xq = np.zeros((N, C, H + 2, W + 2), np.float16)
    xq[:, :, 1:H + 1, 1:W + 1] = x
    # [N, hb, cw, gl, r, col]
    xp = np.empty((N, HB, CW, GL, XR, XW), np.float16)
    xv = xq.reshape(N, CW, GL, H + 2, XW)
    for hb in range(HB):
        xp[:, hb] = xv[:, :, :, hb * RB:hb * RB + XR, :]
    wp = np.asarray(w, np.float16).reshape(N, CW, 9, HB, SB).transpose(
        0, 3, 1, 2, 4)  # [N, hb, cw, k, sb]
    return (np.ascontiguousarray(xp.reshape(N, 128, GL * XGL)),
            np.ascontiguousarray(wp.reshape(N, 128, 9 * SB)))


def unpack_output(o):
    """o: [N,128,8192] f16 -> [N,64,128,128] f32."""
    N = o.shape[0]
    v = o.reshape(N, HB, CW, GL, RB, W).transpose(0, 2, 3, 1, 4, 5)
    return np.ascontiguousarray(v.reshape(N, C, H, W)).astype(np.float32)


def kernel(input, weight):
    """input: [8,64,128,128] f32, weight: [8,8,9,16384] f32 ->
    [8,64,128,128] f32."""
    from concourse.bass_utils import run_bass_kernel_spmd

    x = np.asarray(input, dtype=np.float32)
    w = np.asarray(weight, dtype=np.float32)
    N = x.shape[0]
    xp, wp = pack_inputs(x, w)
    nc = _get_nc()
    in_maps = [{"x": xp[i], "w": wp[i]} for i in range(N)]
    res = run_bass_kernel_spmd(nc, in_maps, core_ids=list(range(N)))
    o = np.stack([res.results[i]["out"] for i in range(N)])
    return unpack_output(o)
